# revision 1
# baseline (speedup 1.0000x reference)
"""CenterLoss kernel for Trainium2 (Bass/Tile), 8-core data-parallel.

loss = sum_i ||x_i - centers[labels_i]||^2
  x: (65536, 512) f32, labels: (65536,) int, centers: (512, 512) f32

Per-core plan (8192 rows each), using the expansion
  loss = sum x^2 - 2*sum_{c,d} S[c,d]*centers[c,d] + sum_c count_c*||C_c||^2
with S = onehot(labels)^T @ x computed on the PE via one-hot matmuls and
count_c precomputed on host (np.bincount of the int labels).

Pipeline per core (hybrid DMA):
  - x chunks 0,1,4,5 stream HBM->SBUF as plain f32 via HWDGE (sync ring,
    contiguous-per-partition layout, ~415 GB/s) and are cast to fp8e4m3
    on DVE tensor_copy (2x mode) / ACT activation-Copy, load-balanced
  - x chunks 2,3 stream via the gpsimd SWDGE casting DMA directly to fp8
    (no on-chip cast work; their sum(x^2) reads fp8, error ~2e-4)
  - small inputs (iota/labf/counts) ride the SWDGE queue, centers the
    scalar HWDGE ring, so nothing shares a completion lane with x chunks
  - GpSimd COMPUTE is never used (Pool ops are 4x slow and lock DVE out
    of its fast SBUF modes); SWDGE descriptor-gen is fine
  - DVE builds one-hot tiles: is_equal(iota_row, label_p)
  - PE: per 256-row group, 4 DoubleRow matmuls accumulate S into PSUM
  - ACT accumulates sum(x^2) per chunk from the f32 data
  - tail: r2 = -2*sum(S.*C) on DVE, r3 = sum_c count_c*||C_c||^2 from
    host-provided counts and on-chip csq; out = r1+r2+r3 as [128,1]
    per-core partials; host sums.
"""

import sys

import numpy as np

sys.path.insert(0, "/opt/trn_rl_repo")

N_CORES = 8
B = 65536
D = 512
B_L = B // N_CORES  # 8192 rows per core

# x chunk sizes in rows (DMA granularity); small edges for pipeline ramp
CHUNK_ROWS = [512, 1536, 2048, 2048, 1792, 256]
assert sum(CHUNK_ROWS) == B_L
NCH = D // 128  # 4 class chunks

_CACHE = {}


def _build():
    """Trace the Bass/Tile program once; returns the compiled Bacc module."""
    if "nc" in _CACHE:
        return _CACHE["nc"]

    import concourse.bacc as bacc
    import concourse.mybir as mybir
    import concourse.tile as tile

    f32 = mybir.dt.float32
    fp8 = mybir.dt.float8e4

    nc = bacc.Bacc("TRN2", debug=False, num_devices=N_CORES)
    x_t = nc.dram_tensor("x", [B_L, D], f32, kind="ExternalInput")
    iota_t = nc.dram_tensor("iota16", [128, D], mybir.dt.float16, kind="ExternalInput")
    labf_t = nc.dram_tensor("labf", [128, B_L // 128], f32, kind="ExternalInput")
    cnt_t = nc.dram_tensor("counts", [128, NCH], f32, kind="ExternalInput")
    c_t = nc.dram_tensor("centers", [D, D], f32, kind="ExternalInput")
    out_t = nc.dram_tensor("out", [128, 1], f32, kind="ExternalOutput")

    with tile.TileContext(nc) as tc:
        with (
            tc.tile_pool(name="misc", bufs=1) as misc_pool,
            tc.tile_pool(name="psum", bufs=1, space="PSUM") as psum_pool,
        ):
            # ALL DMAs go through HWDGE: SWDGE (gpsimd) transfers starve
            # under concurrent HWDGE bulk load (their descriptor rings sit
            # on SBUF ports the x stream hammers; measured 10us+ late).
            # Small inputs first on the scalar ring, x chunks on the sync
            # ring, so the rings work in parallel.
            # small inputs on the SWDGE (gpsimd) queue: under concurrent
            # HWDGE bulk load they complete ~5us later than idle, but that
            # still beats the scalar HWDGE ring, whose completion lanes get
            # sequenced behind multi-MB x chunks (measured +5 to +8us).
            iota_sb = misc_pool.tile([128, D], mybir.dt.float16)
            nc.gpsimd.dma_start(iota_sb[:], iota_t.ap())
            labf_sb = misc_pool.tile([128, B_L // 128], f32)
            nc.gpsimd.dma_start(labf_sb[:], labf_t.ap())
            cnt_sb = misc_pool.tile([128, NCH], f32)
            nc.gpsimd.dma_start(cnt_sb[:], cnt_t.ap())
            cent_sb = misc_pool.tile([128, NCH, D], f32)
            nc.scalar.dma_start(
                cent_sb[:], c_t.ap().rearrange("(n p) d -> p n d", p=128)
            )

            acc_x2 = misc_pool.tile([128, len(CHUNK_ROWS)], f32)
            csq_col = misc_pool.tile([128, NCH], f32)
            junk_dve = misc_pool.tile([128, 1], f32)
            junk_act = misc_pool.tile([128, 1], f32)
            r1 = misc_pool.tile([128, 1], f32)
            r2 = misc_pool.tile([128, 1], f32)
            r3 = misc_pool.tile([128, 1], f32)

            S_all = psum_pool.tile([128, NCH, D], f32, name="S_all")
            S_ps = [S_all[:, c, :] for c in range(NCH)]

            # chunks 3-4 arrive pre-cast to fp8 via SWDGE casting DMA (no
            # on-chip cast work; their sumsq reads fp8); the rest land as
            # f32 via HWDGE and are cast on DVE/ACT
            swdge_chunks = {2, 3}
            # static x tiles per chunk: f32 landing pad + fp8 cast output
            x32 = [
                None
                if i in swdge_chunks
                else misc_pool.tile([128, r // 128, D], f32, name=f"x32_{i}")
                for i, r in enumerate(CHUNK_ROWS)
            ]
            x8 = [
                misc_pool.tile([128, r // 128, D], fp8, name=f"x8_{i}")
                for i, r in enumerate(CHUNK_ROWS)
            ]

            def cast_slab(ci, sl, on_act):
                if on_act:
                    nc.scalar.activation(
                        x8[ci][:, sl, :],
                        x32[ci][:, sl, :],
                        mybir.ActivationFunctionType.Copy,
                    )
                else:
                    nc.vector.tensor_copy(x8[ci][:, sl, :], x32[ci][:, sl, :])

            x_ap = x_t.ap()
            n_chunks = len(CHUNK_ROWS)
            qcs = [r // 128 for r in CHUNK_ROWS]
            toff = [sum(qcs[:i]) for i in range(n_chunks)]  # labf col offset
            goff = [sum(q // 2 for q in qcs[:i]) for i in range(n_chunks)]
            n_groups = B_L // 256  # 32 DoubleRow matmul groups

            slab = 0  # global 512-row slab counter (for cast engine choice)
            lo = 0
            for ci, rows in enumerate(CHUNK_ROWS):
                qc = qcs[ci]
                # contiguous-per-partition layout: partition p holds rows
                # [lo + p*qc, lo + (p+1)*qc)
                src = x_ap[lo : lo + rows, :].rearrange("(p q) d -> p q d", p=128)
                if ci in swdge_chunks:
                    nc.gpsimd.dma_start(x8[ci][:], src)  # cast in flight
                else:
                    nc.sync.dma_start(x32[ci][:], src)
                    # fp8 cast per 512-row slab: DVE 2x mode is cheapest;
                    # every 3rd slab goes to ACT to balance engine load
                    n_sl = (qc + 3) // 4
                    for k in range(n_sl):
                        sl = slice(4 * k, min(4 * k + 4, qc))
                        on_act = (slab % 3 == 2) and ci < n_chunks - 1
                        cast_slab(ci, sl, on_act=on_act)
                        slab += 1
                # per 256-row group: one-hot build + 4 DoubleRow matmuls
                for j in range(qc // 2):
                    oh = misc_pool.tile([128, 2, D], fp8, tag="oh", bufs=12)
                    for u in range(2):
                        tcol = toff[ci] + 2 * j + u
                        nc.vector.tensor_scalar(
                            out=oh[:, u, :],
                            in0=iota_sb[:],
                            scalar1=labf_sb[:, tcol : tcol + 1],
                            scalar2=None,
                            op0=mybir.AluOpType.is_equal,
                        )
                    g = goff[ci] + j
                    first = g == 0
                    last = g == n_groups - 1
                    for c in range(NCH):
                        nc.tensor.matmul(
                            S_ps[c],
                            lhsT=oh[:, :, c * 128 : (c + 1) * 128],
                            rhs=x8[ci][:, 2 * j : 2 * j + 2, :],
                            start=first,
                            stop=last,
                            perf_mode=mybir.MatmulPerfMode.DoubleRow,
                        )
                # sum(x^2) for the whole chunk on ACT (f32, or fp8 for the
                # SWDGE pre-cast chunks — same ACT rate, error ~2e-4 ok)
                x_src = x8[ci] if ci in swdge_chunks else x32[ci]
                x_flat = x_src[:].rearrange("p q d -> p (q d)")
                nc.scalar.activation(
                    junk_act[:].broadcast_to(x_flat.shape),
                    x_flat,
                    mybir.ActivationFunctionType.Square,
                    accum_out=acc_x2[:, ci : ci + 1],
                )
                if ci == 2:
                    # csq[c-chunk] = rowsum(centers^2): mid-stream on ACT
                    # (centers landed long before); consumed by r3 only
                    for c in range(NCH):
                        nc.scalar.activation(
                            junk_act[:].broadcast_to(cent_sb[:, c, :].shape),
                            cent_sb[:, c, :],
                            mybir.ActivationFunctionType.Square,
                            accum_out=csq_col[:, c : c + 1],
                        )
                if ci == 3:
                    # r3 = sum_c count_c * csq_c (host-precomputed histogram)
                    nc.vector.scalar_tensor_tensor(
                        out=junk_dve[:].broadcast_to(cnt_sb[:].shape),
                        in0=cnt_sb[:],
                        scalar=1.0,
                        in1=csq_col[:],
                        op0=mybir.AluOpType.bypass,
                        op1=mybir.AluOpType.mult,
                        accum_out=r3[:],
                    )
                lo += rows

            # tail: r2 = -2*sum_{c,d} S[c,d]*C[c,d] in one fused DVE op
            S_flat = S_all[:].rearrange("p c d -> p (c d)")
            C_flat = cent_sb[:].rearrange("p c d -> p (c d)")
            nc.vector.scalar_tensor_tensor(
                out=junk_dve[:].broadcast_to(S_flat.shape),
                in0=S_flat,
                scalar=-2.0,
                in1=C_flat,
                op0=mybir.AluOpType.mult,
                op1=mybir.AluOpType.mult,
                accum_out=r2[:],
            )
            nc.vector.tensor_reduce(
                r1[:], acc_x2[:], axis=mybir.AxisListType.X, op=mybir.AluOpType.add
            )
            nc.vector.tensor_tensor(r1[:], r1[:], r2[:], op=mybir.AluOpType.add)
            nc.vector.tensor_tensor(r1[:], r1[:], r3[:], op=mybir.AluOpType.add)
            nc.sync.dma_start(out_t.ap(), r1[:])

    nc.compile()
    _CACHE["nc"] = nc
    return nc


def _prep_inputs(x, labels, centers):
    """Shard full inputs into the 8 per-core input maps."""
    x = np.asarray(x, dtype=np.float32)
    labels = np.asarray(labels)
    centers = np.ascontiguousarray(np.asarray(centers, dtype=np.float32))
    iota16 = np.ascontiguousarray(np.tile(np.arange(D, dtype=np.float16), (128, 1)))
    in_maps = []
    for cidx in range(N_CORES):
        xs = np.ascontiguousarray(x[cidx * B_L : (cidx + 1) * B_L])
        lab = np.asarray(labels[cidx * B_L : (cidx + 1) * B_L], dtype=np.int64)
        # labf[p, t]: label of the row that lands at (partition p, q-col t),
        # chunk ci contributing qc = rows/128 q-cols, row = lo + p*qc + qq
        cols = []
        lo = 0
        for rows in CHUNK_ROWS:
            qc = rows // 128
            cols.append(lab[lo : lo + rows].reshape(128, qc))
            lo += rows
        labf = np.ascontiguousarray(np.concatenate(cols, axis=1).astype(np.float32))
        # counts[p, ch] = #{labels == ch*128 + p} (histogram of int indices)
        bc = np.bincount(lab, minlength=D).astype(np.float32)
        counts = np.ascontiguousarray(bc.reshape(NCH, 128).T)
        in_maps.append(
            {
                "x": xs,
                "iota16": iota16,
                "labf": labf,
                "counts": counts,
                "centers": centers,
            }
        )
    return in_maps


def _run(x, labels, centers, trace=False):
    from concourse import bass_utils

    nc = _build()
    in_maps = _prep_inputs(x, labels, centers)
    res = bass_utils.run_bass_kernel_spmd(
        nc, in_maps, core_ids=list(range(N_CORES)), trace=trace
    )
    total = np.float64(0.0)
    for r in res.results:
        total += np.sum(r["out"].astype(np.float64))
    return np.array(total, dtype=np.float32), res


def kernel(x, labels, centers):
    out, _ = _run(x, labels, centers, trace=False)
    return out


def kernel_traced(x, labels, centers):
    return _run(x, labels, centers, trace=True)

